# revision 22
# baseline (speedup 1.0000x reference)
"""Trainium2 Bass kernel for nn_Fields: 16 per-field MLPs (3->16->16->3, ReLU)
applied to 1M points, sharded over 8 NeuronCores along the point axis.

Dataflow per core (points sharded N/8, padded to 4096; slabs of 8192
points, processed slab-at-a-time in three layer phases so each engine's
FIFO order matches true dependency order -- no head-of-line blocking):
  L1: per 512-pt chunk, 2 ROW-TILED concurrent matmuls (K=4 at row groups
      0/32; x replicated at partition stripes 0-3 / 32-35 by two DMAs per
      slab).  Tiled MMs at distinct row groups overlap on silicon (~3x),
      and each tile's weight load hides under the other tile's matmul.
  L2: block-diag [128x128] per field-half, split as 2 COL-TILES of M=64
      (concurrent streams, half-size weight loads).
  L3: 4-way COL-TILED (M=24 at col groups 0/32/64/96); two pairs of
      chunks share one 2-bank PSUM tile so the y evac is one fd=1024 op.
Evacuation is the wall: DVE and ACT both run ~1 elem/cyc/lane from fp32
PSUM (the 2x DVE modes need 2-byte sources, so PSUM reads cannot
accelerate; DMA and GPSIMD have no PSUM access at all).  The greedy
engine split uses silicon-measured per-op costs.  All PSUM rides one
4-buf [128,1024] ring (8 banks); the evac engines pace the ring and PE
chases ~4 tiles ahead.
b1 is folded into L1 via a ones row in x. Output rides a quarter-structured
[128, n_pad/2] bf16 DRAM tensor; the host upcasts and unscrambles.
"""

from contextlib import ExitStack

import ml_dtypes
import numpy as np

import concourse.bass as bass
import concourse.mybir as mybir
import concourse.tile as tile
from concourse import bacc
from concourse.bass_utils import run_bass_kernel_spmd

N_CORES = 8
NFIELDS = 16
HID = 16
C = 512  # chunk = one fp32 PSUM bank of matmul output
PAIR = 2 * C  # 1024 points
GROUP = 4096  # padding unit
SLAB = 8192  # DMA slab (2 groups)

BF16 = mybir.dt.bfloat16
F32 = mybir.dt.float32
AF = mybir.ActivationFunctionType
ALU = mybir.AluOpType

_cache = {}


def build(n_pad, iters=1):
    """Build the per-core Bass program for n_pad points (multiple of GROUP)."""
    assert n_pad % GROUP == 0
    slab_sizes = []
    off = 0
    while off < n_pad:
        s = min(SLAB, n_pad - off)
        slab_sizes.append(s)
        off += s

    nc = bacc.Bacc(None, target_bir_lowering=False)
    xq = nc.declare_dram_parameter("xq", [4, n_pad], BF16, isOutput=False)
    w1q_d = nc.declare_dram_parameter("w1q", [64, 128], BF16, isOutput=False)
    w2s_d = nc.declare_dram_parameter("w2s", [128, 256], BF16, isOutput=False)
    w3s_d = nc.declare_dram_parameter("w3s", [128, 48], BF16, isOutput=False)
    b2r_d = nc.declare_dram_parameter("b2r", [1, 256], BF16, isOutput=False)
    b3r_d = nc.declare_dram_parameter("b3r", [1, 128], BF16, isOutput=False)
    y = nc.declare_dram_parameter("y", [128, n_pad // 2], BF16, isOutput=True)

    with ExitStack() as ctx:
        tc = ctx.enter_context(tile.TileContext(nc))
        consts = ctx.enter_context(tc.tile_pool(name="consts", bufs=1))
        xpool = ctx.enter_context(tc.tile_pool(name="xpool", bufs=3))
        h1pool = ctx.enter_context(tc.tile_pool(name="h1pool", bufs=20))
        h2pool = ctx.enter_context(tc.tile_pool(name="h2pool", bufs=20))
        opool = ctx.enter_context(tc.tile_pool(name="opool", bufs=2))
        psh1 = ctx.enter_context(tc.tile_pool(name="psh1", bufs=4, space="PSUM"))

        w1q = consts.tile([64, 128], BF16)
        nc.sync.dma_start(out=w1q, in_=w1q_d[:, :])
        w2s = consts.tile([128, 256], BF16)
        nc.sync.dma_start(out=w2s, in_=w2s_d[:, :])
        w3s = consts.tile([128, 48], BF16)
        nc.sync.dma_start(out=w3s, in_=w3s_d[:, :])
        b2r = consts.tile([1, 256], BF16)
        nc.sync.dma_start(out=b2r, in_=b2r_d[:, :])
        b3r = consts.tile([1, 128], BF16)
        nc.sync.dma_start(out=b3r, in_=b3r_d[:, :])
        ones_t = consts.tile([1, C], BF16)
        nc.vector.memset(ones_t, 1.0)

        # Greedy engine balancing with silicon-measured evac costs (ns):
        #   ACT: 288 + 1.086*fd (per-partition bias vector costs nothing)
        #   DVE: 125 + 1.12*fd (1-ALU) / 125 + 1.28*fd (2-ALU with bias)
        load = {"act": 0.0, "dve": 0.0}

        def evac(out_ap, in_ap, fd, bias=None, relu=True):
            cost_act = 288 + 1.086 * fd
            cost_dve = 125 + (1.28 if bias is not None else 1.12) * fd
            if load["act"] + cost_act <= load["dve"] + cost_dve:
                load["act"] += cost_act
                nc.scalar.activation(out_ap, in_ap,
                                     AF.Relu if relu else AF.Identity,
                                     bias=0.0 if bias is None else bias)
            else:
                load["dve"] += cost_dve
                if relu:
                    if bias is None:
                        nc.vector.tensor_scalar_max(out_ap, in_ap, 0.0)
                    else:
                        nc.vector.tensor_scalar(out_ap, in_ap, bias, 0.0,
                                                ALU.add, ALU.max)
                else:
                    if bias is None:
                        nc.vector.tensor_copy(out_ap, in_ap)
                    else:
                        nc.vector.tensor_scalar_add(out_ap, in_ap, bias)

        def body(_=None):
            goff = 0
            for g, gsz in enumerate(slab_sizes):
                nch = gsz // C
                npairs = gsz // PAIR
                xsb = xpool.tile([36, SLAB], BF16, tag="xsb", name=f"xsb_{g}")
                for s in range(2):
                    nc.gpsimd.dma_start(out=xsb[32 * s:32 * s + 4, 0:gsz],
                                        in_=xq[0:4, goff:goff + gsz])
                outsb = opool.tile([128, SLAB // 2], BF16, tag="outsb",
                                   name=f"outsb_{g}")

                # ---- phase L1: all chunks, 2 concurrent row-tiled MMs ----
                h1sb = []
                for c in range(nch):
                    hp = psh1.tile([128, PAIR], F32, tag="hps",
                                   name=f"h1ps_{g}_{c}")
                    for s in range(2):
                        nc.tensor.matmul(
                            hp[:, s * C:s * C + C],
                            w1q[32 * s:32 * s + 4, 0:128],
                            xsb[32 * s:32 * s + 4, c * C:c * C + C],
                            start=True, stop=True,
                            tile_position=(32 * s, 0))
                    t = h1pool.tile([128, PAIR], BF16, tag="h1sb",
                                    name=f"h1sb_{g}_{c}")
                    evac(t, hp, PAIR)
                    h1sb.append(t)

                # ---- phase L2: all half-a, then all half-b (col2 tiles) ----
                h2sb = {}
                for hf in range(2):
                    hps = []
                    for p in range(npairs):
                        hp = psh1.tile([128, PAIR], F32, tag="hps",
                                       name=f"h2ps_{g}_{p}_{hf}")
                        for ci in range(2):
                            cc = (2 * p + ci) * C
                            rhs = h1sb[2 * p + ci][:, hf * C:hf * C + C]
                            nc.tensor.matmul(
                                hp[:, ci * C:ci * C + C],
                                b2r[0:1, 128 * hf:128 * hf + 128],
                                ones_t[0:1, 0:C],
                                start=True, stop=False,
                                tile_position=(0, 0), skip_group_check=True)
                            for j in range(2):
                                nc.tensor.matmul(
                                    hp[64 * j:64 * j + 64, ci * C:ci * C + C],
                                    w2s[:, 128 * hf + 64 * j:
                                        128 * hf + 64 * j + 64], rhs,
                                    start=False, stop=(j == 1),
                                    tile_position=(0, 64 * j),
                                    skip_group_check=True)
                        hps.append(hp)
                    for p in range(npairs):
                        t = h2pool.tile([128, PAIR], BF16, tag="h2sb",
                                        name=f"h2sb_{g}_{p}_{hf}")
                        evac(t, hps[p], PAIR)
                        h2sb[(p, hf)] = t

                # ---- phase L3: 4 col-tiled MMs per pair; y evac per
                # 2 pairs (both pairs land in one 2-bank oph tile) ----
                for pq in range(npairs // 2):
                    oph = psh1.tile([128, PAIR], F32, tag="hps",
                                    name=f"oph_{g}_{pq}")
                    for pi in range(2):
                        p = 2 * pq + pi
                        a, b = h2sb[(p, 0)], h2sb[(p, 1)]
                        o = pi * C
                        nc.tensor.matmul(oph[:, o:o + C], b3r[0:1, 0:128],
                                         ones_t[0:1, 0:C],
                                         start=True, stop=False,
                                         tile_position=(0, 0),
                                         skip_group_check=True)
                        nc.tensor.matmul(oph[0:24, o:o + C], w3s[:, 0:24],
                                         a[:, 0:C], start=False, stop=False,
                                         tile_position=(0, 0),
                                         skip_group_check=True)
                        nc.tensor.matmul(oph[32:56, o:o + C], w3s[:, 24:48],
                                         b[:, 0:C], start=False, stop=False,
                                         tile_position=(0, 32),
                                         skip_group_check=True)
                        nc.tensor.matmul(oph[64:88, o:o + C], w3s[:, 0:24],
                                         a[:, C:2 * C], start=False,
                                         stop=False, tile_position=(0, 64),
                                         skip_group_check=True)
                        nc.tensor.matmul(oph[96:120, o:o + C], w3s[:, 24:48],
                                         b[:, C:2 * C], start=False,
                                         stop=True, tile_position=(0, 96),
                                         skip_group_check=True)
                    evac(outsb[0:120, 2 * pq * C:2 * pq * C + PAIR],
                         oph[0:120, 0:PAIR], PAIR, relu=False)
                nc.sync.dma_start(
                    out=y[0:120, goff // 2:goff // 2 + gsz // 2],
                    in_=outsb[0:120, 0:gsz // 2])
                goff += gsz

        if iters == 1:
            body()
        else:
            with tc.For_i(0, iters, 1):
                body()
    nc.finalize()
    return nc


def prep_weights(W1, b1, W2, b2, W3, b3):
    W1 = np.asarray(W1, np.float32); b1 = np.asarray(b1, np.float32)
    W2 = np.asarray(W2, np.float32); b2 = np.asarray(b2, np.float32)
    W3 = np.asarray(W3, np.float32); b3 = np.asarray(b3, np.float32)
    # w1q: stripe s at partitions 32s..32s+3 holds half s of the stacked
    # per-field L1 weights ([x0 x1 x2 1] -> 8 fields x 16 hidden).
    w1q = np.zeros((64, 128), np.float32)
    for half in range(2):
        fb = 8 * half
        for fl in range(8):
            for h in range(HID):
                w1q[32 * half:32 * half + 3, 16 * fl + h] = W1[fb + fl, h, :]
                w1q[32 * half + 3, 16 * fl + h] = b1[fb + fl, h]
    w2s = np.zeros((128, 256), np.float32)
    for half in range(2):
        fb = 8 * half
        for fl in range(8):
            blk = W2[fb + fl]  # [g2, h]
            w2s[16 * fl:16 * fl + 16,
                128 * half + 16 * fl:128 * half + 16 * fl + 16] = blk.T
    w3s = np.zeros((128, 48), np.float32)
    for half in range(2):
        fb = 8 * half
        for fl in range(8):
            blk = W3[fb + fl]  # [o, h]
            w3s[16 * fl:16 * fl + 16,
                24 * half + 3 * fl:24 * half + 3 * fl + 3] = blk.T
    b2r = np.zeros((1, 256), np.float32)
    for half in range(2):
        b2r[0, 128 * half:128 * half + 128] = \
            b2[8 * half:8 * half + 8].reshape(128)
    b3r = np.zeros((1, 128), np.float32)
    for q in range(4):
        fb = 8 * (q % 2)
        b3r[0, 32 * q:32 * q + 24] = b3[fb:fb + 8].reshape(24)
    bf = ml_dtypes.bfloat16
    return {
        "w1q": w1q.astype(bf), "w2s": w2s.astype(bf), "w3s": w3s.astype(bf),
        "b2r": b2r.astype(bf), "b3r": b3r.astype(bf),
    }


def _get_nc(n_pad, iters=1):
    key = (n_pad, iters)
    if key not in _cache:
        _cache[key] = build(n_pad, iters)
    return _cache[key]


def run(x_np, weights, n_pad, iters=1, n=None):
    """x_np: [3, N] f32 full; returns [16, 3, N] f32."""
    if n is None:
        n = x_np.shape[1]
    assert n % N_CORES == 0
    npc = n // N_CORES
    assert npc <= n_pad
    nc = _get_nc(n_pad, iters)
    bf = ml_dtypes.bfloat16
    in_maps = []
    for c in range(N_CORES):
        xs = np.zeros((4, n_pad), np.float32)
        xs[0:3, :npc] = x_np[:, c * npc:(c + 1) * npc]
        xs[3, :] = 1.0
        in_maps.append({"xq": xs.astype(bf), **weights})
    res = run_bass_kernel_spmd(nc, in_maps, core_ids=list(range(N_CORES)))
    out = np.empty((NFIELDS, 3, n), np.float32)
    npr = n_pad // PAIR
    for c in range(N_CORES):
        yc = np.asarray(res.results[c]["y"], np.float32)  # [128, n_pad//2]
        yv = yc.reshape(128, npr, C)
        oc = np.empty((NFIELDS, 3, npr, 2, C), np.float32)
        for q in range(4):
            blk = yv[32 * q:32 * q + 24].reshape(8, 3, npr, C)
            oc[8 * (q % 2):8 * (q % 2) + 8, :, :, q // 2, :] = blk
        out[:, :, c * npc:(c + 1) * npc] = \
            oc.reshape(NFIELDS, 3, n_pad)[:, :, :npc]
    return out


def kernel(x, W1, b1, W2, b2, W3, b3, D):
    x = np.asarray(x, np.float32)
    n = x.shape[2]
    npc = n // N_CORES
    n_pad = ((npc + GROUP - 1) // GROUP) * GROUP
    weights = prep_weights(W1, b1, W2, b2, W3, b3)
    return run(x[0], weights, n_pad)


# revision 23
# speedup vs baseline: 2.1554x; 2.1554x over previous
"""Trainium2 Bass kernel for nn_Fields: 16 per-field MLPs (3->16->16->3, ReLU)
applied to 1M points, sharded over 8 NeuronCores along the point axis.

Dataflow per core (points sharded N/8, padded to 4096; slabs of 8192
points, processed slab-at-a-time in three layer phases so each engine's
FIFO order matches true dependency order -- no head-of-line blocking):
  L1: per 512-pt chunk, 2 ROW-TILED concurrent matmuls (K=4 at row groups
      0/32; x replicated at partition stripes 0-3 / 32-35 by two DMAs per
      slab).  Tiled MMs at distinct row groups overlap on silicon (~3x),
      and each tile's weight load hides under the other tile's matmul.
  L2: block-diag [128x128] per field-half, split as 2 COL-TILES of M=64
      (concurrent streams, half-size weight loads).
  L3: 4-way COL-TILED (M=24 at col groups 0/32/64/96); two pairs of
      chunks share one 2-bank PSUM tile so the y evac is one fd=1024 op.
Evacuation is the wall: DVE and ACT both run ~1 elem/cyc/lane from fp32
PSUM (the 2x DVE modes need 2-byte sources, so PSUM reads cannot
accelerate; DMA and GPSIMD have no PSUM access at all).  The greedy
engine split uses silicon-measured per-op costs.  All PSUM rides one
4-buf [128,1024] ring (8 banks); the evac engines pace the ring and PE
chases ~4 tiles ahead.
b1 is folded into L1 via a ones row in x. Output rides a quarter-structured
[128, n_pad/2] bf16 DRAM tensor; the host upcasts and unscrambles.
"""

from contextlib import ExitStack

import ml_dtypes
import numpy as np

import concourse.bass as bass
import concourse.mybir as mybir
import concourse.tile as tile
from concourse import bacc
from concourse.bass_utils import run_bass_kernel_spmd

N_CORES = 8
NFIELDS = 16
HID = 16
C = 512  # chunk = one fp32 PSUM bank of matmul output
PAIR = 2 * C  # 1024 points
GROUP = 4096  # padding unit
SLAB = 8192  # DMA slab (2 groups)

BF16 = mybir.dt.bfloat16
F32 = mybir.dt.float32
AF = mybir.ActivationFunctionType
ALU = mybir.AluOpType

_cache = {}


def build(n_pad, iters=1):
    """Build the per-core Bass program for n_pad points (multiple of GROUP)."""
    assert n_pad % GROUP == 0
    slab_sizes = []
    off = 0
    while off < n_pad:
        s = min(SLAB, n_pad - off)
        slab_sizes.append(s)
        off += s

    nc = bacc.Bacc(None, target_bir_lowering=False)
    xq = nc.declare_dram_parameter("xq", [4, n_pad], BF16, isOutput=False)
    w1q_d = nc.declare_dram_parameter("w1q", [64, 128], BF16, isOutput=False)
    w2s_d = nc.declare_dram_parameter("w2s", [128, 256], BF16, isOutput=False)
    w3s_d = nc.declare_dram_parameter("w3s", [128, 48], BF16, isOutput=False)
    b2v_d = nc.declare_dram_parameter("b2v", [128, 2], F32, isOutput=False)
    b3v_d = nc.declare_dram_parameter("b3v", [128, 1], F32, isOutput=False)
    y = nc.declare_dram_parameter("y", [128, n_pad // 2], BF16, isOutput=True)

    with ExitStack() as ctx:
        tc = ctx.enter_context(tile.TileContext(nc))
        consts = ctx.enter_context(tc.tile_pool(name="consts", bufs=1))
        xpool = ctx.enter_context(tc.tile_pool(name="xpool", bufs=3))
        h1pool = ctx.enter_context(tc.tile_pool(name="h1pool", bufs=20))
        h2pool = ctx.enter_context(tc.tile_pool(name="h2pool", bufs=20))
        opool = ctx.enter_context(tc.tile_pool(name="opool", bufs=2))
        psh1 = ctx.enter_context(tc.tile_pool(name="psh1", bufs=4, space="PSUM"))

        w1q = consts.tile([64, 128], BF16)
        nc.sync.dma_start(out=w1q, in_=w1q_d[:, :])
        w2s = consts.tile([128, 256], BF16)
        nc.sync.dma_start(out=w2s, in_=w2s_d[:, :])
        w3s = consts.tile([128, 48], BF16)
        nc.sync.dma_start(out=w3s, in_=w3s_d[:, :])
        b2v = consts.tile([128, 2], F32)
        nc.sync.dma_start(out=b2v, in_=b2v_d[:, :])
        b3v = consts.tile([128, 1], F32)
        nc.sync.dma_start(out=b3v, in_=b3v_d[:, :])

        # Greedy engine balancing with silicon-measured evac costs (ns):
        #   ACT: 288 + 1.086*fd (per-partition bias vector costs nothing)
        #   DVE: 125 + 1.12*fd (1-ALU) / 125 + 1.28*fd (2-ALU with bias)
        load = {"act": 0.0, "dve": 0.0}

        def evac(out_ap, in_ap, fd, bias=None, relu=True):
            cost_act = 288 + 1.086 * fd
            cost_dve = 125 + (1.28 if bias is not None else 1.12) * fd
            if load["act"] + cost_act <= load["dve"] + cost_dve:
                load["act"] += cost_act
                nc.scalar.activation(out_ap, in_ap,
                                     AF.Relu if relu else AF.Identity,
                                     bias=0.0 if bias is None else bias)
            else:
                load["dve"] += cost_dve
                if relu:
                    if bias is None:
                        nc.vector.tensor_scalar_max(out_ap, in_ap, 0.0)
                    else:
                        nc.vector.tensor_scalar(out_ap, in_ap, bias, 0.0,
                                                ALU.add, ALU.max)
                else:
                    if bias is None:
                        nc.vector.tensor_copy(out_ap, in_ap)
                    else:
                        nc.vector.tensor_scalar_add(out_ap, in_ap, bias)

        def body(_=None):
            goff = 0
            for g, gsz in enumerate(slab_sizes):
                nch = gsz // C
                npairs = gsz // PAIR
                xsb = xpool.tile([36, SLAB], BF16, tag="xsb", name=f"xsb_{g}")
                for s in range(2):
                    nc.gpsimd.dma_start(out=xsb[32 * s:32 * s + 4, 0:gsz],
                                        in_=xq[0:4, goff:goff + gsz])
                outsb = opool.tile([128, SLAB // 2], BF16, tag="outsb",
                                   name=f"outsb_{g}")

                # ---- phase L1: all chunks, 2 concurrent row-tiled MMs ----
                h1sb = []
                for c in range(nch):
                    hp = psh1.tile([128, PAIR], F32, tag="hps",
                                   name=f"h1ps_{g}_{c}")
                    for s in range(2):
                        nc.tensor.matmul(
                            hp[:, s * C:s * C + C],
                            w1q[32 * s:32 * s + 4, 0:128],
                            xsb[32 * s:32 * s + 4, c * C:c * C + C],
                            start=True, stop=True,
                            tile_position=(32 * s, 0))
                    t = h1pool.tile([128, PAIR], BF16, tag="h1sb",
                                    name=f"h1sb_{g}_{c}")
                    evac(t, hp, PAIR)
                    h1sb.append(t)

                # ---- phase L2: all half-a, then all half-b (col2 tiles) ----
                h2sb = {}
                for hf in range(2):
                    hps = []
                    for p in range(npairs):
                        hp = psh1.tile([128, PAIR], F32, tag="hps",
                                       name=f"h2ps_{g}_{p}_{hf}")
                        for ci in range(2):
                            rhs = h1sb[2 * p + ci][:, hf * C:hf * C + C]
                            for j in range(2):
                                nc.tensor.matmul(
                                    hp[64 * j:64 * j + 64, ci * C:ci * C + C],
                                    w2s[:, 128 * hf + 64 * j:
                                        128 * hf + 64 * j + 64], rhs,
                                    start=True, stop=True,
                                    tile_position=(0, 64 * j))
                        hps.append(hp)
                    for p in range(npairs):
                        t = h2pool.tile([128, PAIR], BF16, tag="h2sb",
                                        name=f"h2sb_{g}_{p}_{hf}")
                        evac(t, hps[p], PAIR, bias=b2v[:, hf:hf + 1])
                        h2sb[(p, hf)] = t

                # ---- phase L3: 4 col-tiled MMs per pair; y evac per
                # 2 pairs (both pairs land in one 2-bank oph tile) ----
                for pq in range(npairs // 2):
                    oph = psh1.tile([128, PAIR], F32, tag="hps",
                                    name=f"oph_{g}_{pq}")
                    for pi in range(2):
                        p = 2 * pq + pi
                        a, b = h2sb[(p, 0)], h2sb[(p, 1)]
                        o = pi * C
                        nc.tensor.matmul(oph[0:24, o:o + C], w3s[:, 0:24],
                                         a[:, 0:C], start=True, stop=True,
                                         tile_position=(0, 0))
                        nc.tensor.matmul(oph[32:56, o:o + C], w3s[:, 24:48],
                                         b[:, 0:C], start=True, stop=True,
                                         tile_position=(0, 32))
                        nc.tensor.matmul(oph[64:88, o:o + C], w3s[:, 0:24],
                                         a[:, C:2 * C], start=True, stop=True,
                                         tile_position=(0, 64))
                        nc.tensor.matmul(oph[96:120, o:o + C], w3s[:, 24:48],
                                         b[:, C:2 * C], start=True, stop=True,
                                         tile_position=(0, 96))
                    evac(outsb[0:120, 2 * pq * C:2 * pq * C + PAIR],
                         oph[0:120, 0:PAIR], PAIR,
                         bias=b3v[0:120, 0:1], relu=False)
                nc.sync.dma_start(
                    out=y[0:120, goff // 2:goff // 2 + gsz // 2],
                    in_=outsb[0:120, 0:gsz // 2])
                goff += gsz

        if iters == 1:
            body()
        else:
            with tc.For_i(0, iters, 1):
                body()
    nc.finalize()
    return nc


def prep_weights(W1, b1, W2, b2, W3, b3):
    W1 = np.asarray(W1, np.float32); b1 = np.asarray(b1, np.float32)
    W2 = np.asarray(W2, np.float32); b2 = np.asarray(b2, np.float32)
    W3 = np.asarray(W3, np.float32); b3 = np.asarray(b3, np.float32)
    # w1q: stripe s at partitions 32s..32s+3 holds half s of the stacked
    # per-field L1 weights ([x0 x1 x2 1] -> 8 fields x 16 hidden).
    w1q = np.zeros((64, 128), np.float32)
    for half in range(2):
        fb = 8 * half
        for fl in range(8):
            for h in range(HID):
                w1q[32 * half:32 * half + 3, 16 * fl + h] = W1[fb + fl, h, :]
                w1q[32 * half + 3, 16 * fl + h] = b1[fb + fl, h]
    w2s = np.zeros((128, 256), np.float32)
    for half in range(2):
        fb = 8 * half
        for fl in range(8):
            blk = W2[fb + fl]  # [g2, h]
            w2s[16 * fl:16 * fl + 16,
                128 * half + 16 * fl:128 * half + 16 * fl + 16] = blk.T
    w3s = np.zeros((128, 48), np.float32)
    for half in range(2):
        fb = 8 * half
        for fl in range(8):
            blk = W3[fb + fl]  # [o, h]
            w3s[16 * fl:16 * fl + 16,
                24 * half + 3 * fl:24 * half + 3 * fl + 3] = blk.T
    b2v = np.zeros((128, 2), np.float32)
    for half in range(2):
        b2v[:, half] = b2[8 * half:8 * half + 8].reshape(128)
    b3v = np.zeros((128, 1), np.float32)
    for q in range(4):
        fb = 8 * (q % 2)
        b3v[32 * q:32 * q + 24, 0] = b3[fb:fb + 8].reshape(24)
    bf = ml_dtypes.bfloat16
    return {
        "w1q": w1q.astype(bf), "w2s": w2s.astype(bf), "w3s": w3s.astype(bf),
        "b2v": b2v, "b3v": b3v,
    }


def _get_nc(n_pad, iters=1):
    key = (n_pad, iters)
    if key not in _cache:
        _cache[key] = build(n_pad, iters)
    return _cache[key]


def run(x_np, weights, n_pad, iters=1, n=None):
    """x_np: [3, N] f32 full; returns [16, 3, N] f32."""
    if n is None:
        n = x_np.shape[1]
    assert n % N_CORES == 0
    npc = n // N_CORES
    assert npc <= n_pad
    nc = _get_nc(n_pad, iters)
    bf = ml_dtypes.bfloat16
    in_maps = []
    for c in range(N_CORES):
        xs = np.zeros((4, n_pad), np.float32)
        xs[0:3, :npc] = x_np[:, c * npc:(c + 1) * npc]
        xs[3, :] = 1.0
        in_maps.append({"xq": xs.astype(bf), **weights})
    res = run_bass_kernel_spmd(nc, in_maps, core_ids=list(range(N_CORES)))
    out = np.empty((NFIELDS, 3, n), np.float32)
    npr = n_pad // PAIR
    for c in range(N_CORES):
        yc = np.asarray(res.results[c]["y"], np.float32)  # [128, n_pad//2]
        yv = yc.reshape(128, npr, C)
        oc = np.empty((NFIELDS, 3, npr, 2, C), np.float32)
        for q in range(4):
            blk = yv[32 * q:32 * q + 24].reshape(8, 3, npr, C)
            oc[8 * (q % 2):8 * (q % 2) + 8, :, :, q // 2, :] = blk
        out[:, :, c * npc:(c + 1) * npc] = \
            oc.reshape(NFIELDS, 3, n_pad)[:, :, :npc]
    return out


def kernel(x, W1, b1, W2, b2, W3, b3, D):
    x = np.asarray(x, np.float32)
    n = x.shape[2]
    npc = n // N_CORES
    n_pad = ((npc + GROUP - 1) // GROUP) * GROUP
    weights = prep_weights(W1, b1, W2, b2, W3, b3)
    return run(x[0], weights, n_pad)
